# revision 1
# baseline (speedup 1.0000x reference)
"""KNN-Attention Trainium2 kernel (Bass/Tile), SPMD over 8 NeuronCores.

Problem (nn_KNNAttention): B=2, H=8, S=2048, D=64, K=32.
  q:[B,H,S,D] k,v:[B,S,D] mask:[B,S] mem_k,mem_v:[B,H,S,K,D]
  mem_mask:[B,H,S,K] rel_pos_bias:[1,H,S,S] scale:[H,1,1]
  out[b,h,i,:] = softmax([sim_mem | sim_local]) @ [mem_v | v]

Sharding: data-parallel over B x tensor-parallel over H.
core c -> (b = c//4, heads 2*(c%4), 2*(c%4)+1). k/v/mask replicated per b.

Per-core dataflow (2 heads x 16 i-tiles of 128 tokens):
  - l2norm(k) once -> kT [64, 2048] resident (PE transpose).
  - v' = [v*mask | mask] bf16 resident; the extra column yields the local
    softmax denominator from the same matmul that computes attn@v.
  - per (head, i-tile):
      qs = q * exp(scale)/||q||  (scale folded into q)
      scores = qsT.T @ kT (fp32 PE) -> +bias (DVE) -> exp (ACT, bf16 out)
      exp blocks PE-transposed -> AV matmul (bf16) accumulates [i, 65] psum
      mem: prod = mem_k*qs (GPSIMD) -> seg-reduce d (DVE) -> exp (ACT)
           prod2 = mem_v*exp_mem (GPSIMD/DVE split) -> seg-reduce kk (DVE)
      out = (local_num + mem_num) / (local_den + mem_den)
  - causal handled by only computing j<=i blocks; the upper triangle of the
    diagonal bias blocks is set to -FLT_MAX host-side (exp -> 0, exact).
"""

import os
import sys
from contextlib import ExitStack

import numpy as np

sys.path.insert(0, "/opt/trn_rl_repo")

import concourse.bass as bass
import concourse.mybir as mybir
import concourse.tile as tile
from concourse import bacc

# Keep all ACT functions in ONE table set (natural_log_exp_and_others holds
# Exp+Ln+Copy+Identity) so the kernel pays a single ACT_TABLE_LOAD instead of
# swapping sets every iteration. Other sets keep their dict position (the
# act_func_set_id is positional) but lose the overlapping functions, forcing
# the selector to the combined set.
_orig_get_act_tables = bacc.get_activation_tables
_PREF_SET = "natural_log_exp_and_others"


def _uni_act_tables(arch):
    tabs = _orig_get_act_tables(arch)
    if _PREF_SET in tabs:
        pref = tabs[_PREF_SET]
        for name, funcs in tabs.items():
            if name != _PREF_SET:
                tabs[name] = funcs - pref
    return tabs


bacc.get_activation_tables = _uni_act_tables
from concourse.bass_utils import run_bass_kernel_spmd

B, H, S, D, KK = 2, 8, 2048, 64, 32
P = 128
NT = S // P  # 16 i-tiles
NH = 2  # heads per core
N_CORES = 8
NEG = -np.finfo(np.float32).max
import ml_dtypes

IDENT_F = np.eye(P, dtype=np.float32)
IDENT_B = np.eye(P, dtype=np.float32).astype(ml_dtypes.bfloat16)

F32 = mybir.dt.float32
BF16 = mybir.dt.bfloat16
U8 = mybir.dt.uint8
AX = mybir.AxisListType
ALU = mybir.AluOpType
ACTF = mybir.ActivationFunctionType


def build_program(nh=NH, nt=NT):
    """Build the per-core Bass program (SPMD: same program, different data)."""
    nc = bacc.Bacc("TRN2")
    s = nt * P

    q_d = nc.dram_tensor("q", [nh, s, D], F32, kind="ExternalInput")
    k_d = nc.dram_tensor("k", [s, D], F32, kind="ExternalInput")
    v_d = nc.dram_tensor("v", [s, D], F32, kind="ExternalInput")
    mask_d = nc.dram_tensor("mask", [s], F32, kind="ExternalInput")
    memk_d = nc.dram_tensor("mem_k", [nh, s, KK, D], F32, kind="ExternalInput")
    memv_d = nc.dram_tensor("mem_v", [nh, s, KK, D], F32, kind="ExternalInput")
    mmask_d = nc.dram_tensor("mem_mask", [nh, s, KK], U8, kind="ExternalInput")
    bias_d = nc.dram_tensor("bias", [nh, s, s], BF16, kind="ExternalInput")
    scale_d = nc.dram_tensor("scale", [nh], F32, kind="ExternalInput")
    identf_d = nc.dram_tensor("ident_f", [P, P], F32, kind="ExternalInput")
    identb_d = nc.dram_tensor("ident_b", [P, P], BF16, kind="ExternalInput")
    out_d = nc.dram_tensor("out", [nh, s, D], F32, kind="ExternalOutput")

    with tile.TileContext(nc) as tc, ExitStack() as ctx:
        const = ctx.enter_context(tc.tile_pool(name="const", bufs=1))
        setup = ctx.enter_context(tc.tile_pool(name="setup", bufs=3))
        qpool = ctx.enter_context(tc.tile_pool(name="qpool", bufs=4))
        stream = ctx.enter_context(tc.tile_pool(name="stream", bufs=3))
        work = ctx.enter_context(tc.tile_pool(name="work", bufs=6))
        expTp = ctx.enter_context(tc.tile_pool(name="expTp", bufs=8))
        memw = ctx.enter_context(tc.tile_pool(name="memw", bufs=3))
        smallw = ctx.enter_context(tc.tile_pool(name="smallw", bufs=8))
        outp = ctx.enter_context(tc.tile_pool(name="outp", bufs=2))
        ps_sco = ctx.enter_context(tc.tile_pool(name="ps_sco", bufs=2, space="PSUM"))
        ps_tp_f = ctx.enter_context(tc.tile_pool(name="ps_tp_f", bufs=2, space="PSUM"))
        ps_tp_b = ctx.enter_context(tc.tile_pool(name="ps_tp_b", bufs=2, space="PSUM"))
        ps_u = ctx.enter_context(tc.tile_pool(name="ps_u", bufs=2, space="PSUM"))

        # ---- constants (DMA'd: keeps PE instruction wait lists short) ----
        ident_f = const.tile([P, P], F32)
        nc.sync.dma_start(out=ident_f, in_=identf_d[:])
        ident_b = const.tile([P, P], BF16)
        nc.sync.dma_start(out=ident_b, in_=identb_d[:])

        # ---- sc[h] = exp(scale[h]) broadcast to [P,1] per head via DMA ----
        sc_b = const.tile([P, nh], F32)
        sc_raw = const.tile([P, nh], F32)
        nc.sync.dma_start(
            out=sc_raw, in_=scale_d[None, :].to_broadcast((P, nh))
        )
        nc.scalar.activation(sc_b, sc_raw, ACTF.Exp)

        # ---- k: l2norm, transpose -> kT [64, s]; v' = [v*mask | mask] bf16 ----
        kT_stage = const.tile([D, s], F32)
        kT = const.tile([D, s], F32)
        v_bf = const.tile([P, nt, D + 1], BF16)
        for jt in range(nt):
            k_t = setup.tile([P, D], F32, tag="k_t")
            nc.sync.dma_start(out=k_t, in_=k_d[jt * P : (jt + 1) * P, :])
            ksq = setup.tile([P, D], F32, tag="ksq")
            nc.vector.tensor_mul(ksq, k_t, k_t)
            ksum = setup.tile([P, 1], F32, tag="ksum")
            nc.vector.tensor_reduce(ksum, ksq, axis=AX.X, op=ALU.add)
            kln = setup.tile([P, 1], F32, tag="kln")
            nc.scalar.activation(kln, ksum, ACTF.Ln)
            # rsqrt(sumsq) = exp(-0.5*ln(sumsq)); Ln+Exp share one ACT table set
            rk = setup.tile([P, 1], F32, tag="rk")
            nc.scalar.activation(rk, kln, ACTF.Exp, scale=-0.5)
            kn = setup.tile([P, D], F32, tag="kn")
            nc.vector.tensor_scalar_mul(kn, k_t, rk)
            ps_k = ps_tp_f.tile([D, P], F32, tag="tpf")
            nc.tensor.transpose(ps_k, kn, ident_f)
            nc.scalar.copy(kT_stage[:, jt * P : (jt + 1) * P], ps_k)
        # single-writer consolidation so matmuls reading kT wait on one proc
        nc.vector.tensor_copy(kT, kT_stage)

        # v' built with two instructions total (writer-count discipline)
        v_sb = const.tile([P, nt, D], F32)
        nc.sync.dma_start(
            out=v_sb, in_=v_d[:].rearrange("(t p) d -> p t d", p=P)
        )
        m_sb = const.tile([P, nt], F32)
        nc.sync.dma_start(out=m_sb, in_=mask_d[:].rearrange("(t p) -> p t", p=P))
        nc.vector.tensor_tensor(
            v_bf[:, :, 0:D], v_sb, m_sb[:, :, None].to_broadcast((P, nt, D)), ALU.mult
        )
        nc.vector.tensor_copy(v_bf[:, :, D], m_sb)

        # ---- main loop ----
        for h in range(nh):
            out_acc = outp.tile([P, nt, D], F32, tag="out_acc")
            for it in range(nt):
                jext = (it + 1) * P
                # q tile: l2norm and fold in sc
                q_t = qpool.tile([P, D], F32, tag="q_t")
                nc.sync.dma_start(out=q_t, in_=q_d[h, it * P : (it + 1) * P, :])
                qsq = qpool.tile([P, D], F32, tag="qsq")
                nc.gpsimd.tensor_mul(qsq, q_t, q_t)
                qsum = qpool.tile([P, 1], F32, tag="qsum")
                nc.vector.tensor_reduce(qsum, qsq, axis=AX.X, op=ALU.add)
                qln = qpool.tile([P, 1], F32, tag="qln")
                nc.scalar.activation(qln, qsum, ACTF.Ln)
                rq = qpool.tile([P, 1], F32, tag="rq")
                nc.scalar.activation(rq, qln, ACTF.Exp, scale=-0.5)
                sc_rq = qpool.tile([P, 1], F32, tag="sc_rq")
                nc.vector.tensor_mul(sc_rq, rq, sc_b[:, h : h + 1])
                qs = qpool.tile([P, D], F32, tag="qs")
                nc.vector.tensor_scalar_mul(qs, q_t, sc_rq)
                ps_q = ps_tp_f.tile([D, P], F32, tag="tpf")
                nc.tensor.transpose(ps_q, qs, ident_f)
                qT = qpool.tile([D, P], F32, tag="qT")
                nc.scalar.copy(qT, ps_q)

                # streamed tiles
                memk = stream.tile([P, KK, D], F32, tag="memk")
                nc.sync.dma_start(out=memk, in_=memk_d[h, it * P : (it + 1) * P])
                memv = stream.tile([P, KK, D], F32, tag="memv")
                nc.sync.dma_start(out=memv, in_=memv_d[h, it * P : (it + 1) * P])
                mmask = stream.tile([P, KK], U8, tag="mmask")
                nc.sync.dma_start(out=mmask, in_=mmask_d[h, it * P : (it + 1) * P])
                bias_t = stream.tile([P, S], BF16, tag="bias_t")
                nc.sync.dma_start(
                    out=bias_t[:, :jext],
                    in_=bias_d[h, it * P : (it + 1) * P, 0:jext],
                )

                # ---- knn-memory branch ----
                prod = memw.tile([P, KK, D], F32, tag="prod")
                nc.gpsimd.tensor_tensor(
                    prod, memk, qs[:, None, :].to_broadcast((P, KK, D)), ALU.mult
                )
                simmem = smallw.tile([P, KK], F32, tag="simmem")
                nc.vector.tensor_reduce(simmem, prod, axis=AX.X, op=ALU.add)
                # joint-softmax stabilizer: M = max(rowmax(sim_mem), 21) covers
                # the unnormalized mem logits (~N(0,20)); local logits are
                # bounded by 20+|bias| < 21, so exp(l - M) never overflows.
                rowmax = smallw.tile([P, 1], F32, tag="rowmax")
                nc.vector.tensor_reduce(rowmax, simmem, axis=AX.X, op=ALU.max)
                negM = smallw.tile([P, 1], F32, tag="negM")
                nc.vector.tensor_scalar(
                    negM, rowmax, 21.0, -1.0, ALU.max, ALU.mult
                )
                expmem = smallw.tile([P, KK], F32, tag="expmem")
                nc.scalar.activation(expmem, simmem, ACTF.Exp, bias=negM)
                mmf = smallw.tile([P, KK], F32, tag="mmf")
                nc.gpsimd.tensor_copy(mmf, mmask)
                nc.gpsimd.tensor_mul(expmem, expmem, mmf)
                zmem = smallw.tile([P, 1], F32, tag="zmem")
                nc.vector.tensor_reduce(zmem, expmem, axis=AX.X, op=ALU.add)
                prod2 = memw.tile([P, D, KK], F32, tag="prod2")
                p2w = prod2[:].rearrange("p d k -> p k d")
                eb = expmem[:, :, None].to_broadcast((P, KK, D))
                nc.gpsimd.tensor_tensor(p2w, memv, eb, ALU.mult)
                memout = smallw.tile([P, D], F32, tag="memout")
                nc.vector.tensor_reduce(memout, prod2, axis=AX.X, op=ALU.add)

                # ---- local branch ----
                psum_u = ps_u.tile([P, D + 1], F32, tag="u")
                for j0 in range(0, jext, 512):
                    w = min(512, jext - j0)
                    ps_s = ps_sco.tile([P, 512], F32, tag="sco")
                    nc.tensor.matmul(
                        ps_s[:, :w],
                        lhsT=qT,
                        rhs=kT[:, j0 : j0 + w],
                        start=True,
                        stop=True,
                    )
                    expb0 = work.tile([P, 512], BF16, tag="expb0")
                    nc.scalar.activation(expb0[:, :w], ps_s[:, :w], ACTF.Exp, bias=negM)
                    expb = work.tile([P, 512], BF16, tag="expb")
                    nc.vector.tensor_mul(
                        expb[:, :w], expb0[:, :w], bias_t[:, j0 : j0 + w]
                    )
                    for jj in range(0, w, P):
                        jt_g = (j0 + jj) // P
                        ps_t = ps_tp_b.tile([P, P], BF16, tag="tpb")
                        nc.tensor.transpose(ps_t, expb[:, jj : jj + P], ident_b)
                        eT = expTp.tile([P, P], BF16, tag="eT")
                        nc.scalar.copy(eT, ps_t)
                        nc.tensor.matmul(
                            psum_u,
                            lhsT=eT,
                            rhs=v_bf[:, jt_g, :],
                            start=(jt_g == 0),
                            stop=(jt_g == it),
                        )

                # ---- combine ----
                num = smallw.tile([P, D], F32, tag="num")
                nc.vector.tensor_add(num, psum_u[:, 0:D], memout)
                z = smallw.tile([P, 1], F32, tag="z")
                nc.vector.tensor_add(z, psum_u[:, D : D + 1], zmem)
                rz = smallw.tile([P, 1], F32, tag="rz")
                nc.vector.reciprocal(rz, z)
                nc.vector.tensor_scalar_mul(out_acc[:, it, :], num, rz)

            nc.sync.dma_start(
                out=out_d[h].rearrange("(t p) d -> p t d", p=P), in_=out_acc
            )

    nc.compile()
    return nc


_CACHED = {}
TRACE = False
TRACE_CORES = [0]
STITCH = False
LAST_RESULTS = None


def _get_program(nh=NH, nt=NT):
    key = (nh, nt)
    if key not in _CACHED:
        _CACHED[key] = build_program(nh, nt)
    return _CACHED[key]


def _merge_causal(bias):
    """bias: [H, S, S] float32 (a copy). Set upper triangle of each diagonal
    128-block to -FLT_MAX. Off-diagonal upper blocks are never read."""
    iu = np.triu_indices(P, 1)
    for t in range(S // P):
        blk = bias[:, t * P : (t + 1) * P, t * P : (t + 1) * P]
        blk[:, iu[0], iu[1]] = NEG
    return bias


def kernel(**inputs):
    q = np.ascontiguousarray(inputs["q"], dtype=np.float32)
    k = np.ascontiguousarray(inputs["k"], dtype=np.float32)
    v = np.ascontiguousarray(inputs["v"], dtype=np.float32)
    mask = np.ascontiguousarray(inputs["mask"], dtype=np.float32)
    mem_k = np.ascontiguousarray(inputs["mem_k"], dtype=np.float32)
    mem_v = np.ascontiguousarray(inputs["mem_v"], dtype=np.float32)
    mem_mask = np.ascontiguousarray(inputs["mem_mask"]).astype(np.uint8)
    rel_pos_bias = np.array(inputs["rel_pos_bias"], dtype=np.float32)
    scale = np.ascontiguousarray(inputs["scale"], dtype=np.float32).reshape(H)

    bias = _merge_causal(rel_pos_bias.reshape(H, S, S).copy())
    bias = np.exp(bias).astype(ml_dtypes.bfloat16)

    nc = _get_program()
    in_maps = []
    for c in range(N_CORES):
        b = c // 4
        h0 = 2 * (c % 4)
        in_maps.append(
            {
                "q": np.ascontiguousarray(q[b, h0 : h0 + NH]),
                "k": k[b],
                "v": v[b],
                "mask": mask[b],
                "mem_k": np.ascontiguousarray(mem_k[b, h0 : h0 + NH]),
                "mem_v": np.ascontiguousarray(mem_v[b, h0 : h0 + NH]),
                "mem_mask": np.ascontiguousarray(mem_mask[b, h0 : h0 + NH]),
                "bias": np.ascontiguousarray(bias[h0 : h0 + NH]),
                "scale": np.ascontiguousarray(scale[h0 : h0 + NH]),
                "ident_f": IDENT_F,
                "ident_b": IDENT_B,
            }
        )

    global LAST_RESULTS
    kwargs = {}
    if TRACE:
        kwargs.update(trace=True, trace_cores=TRACE_CORES, stitch_traces=STITCH)
    res = run_bass_kernel_spmd(nc, in_maps, core_ids=list(range(N_CORES)), **kwargs)
    LAST_RESULTS = res

    out = np.zeros((B, H, S, D), np.float32)
    for c in range(N_CORES):
        b = c // 4
        h0 = 2 * (c % 4)
        out[b, h0 : h0 + NH] = res.results[c]["out"]
    return out


if __name__ == "__main__":
    # smoke test via CoreSim on a reduced config
    from concourse.bass_interp import CoreSim

    nh, nt = int(os.environ.get("SMOKE_NH","1")), int(os.environ.get("SMOKE_NT","2"))
    s = nt * P
    rng = np.random.default_rng(0)
    qs = rng.standard_normal((nh, s, D), dtype=np.float32)
    ks = rng.standard_normal((s, D), dtype=np.float32)
    vs = rng.standard_normal((s, D), dtype=np.float32)
    ms = np.ones((s,), np.float32)
    mks = rng.standard_normal((nh, s, KK, D), dtype=np.float32)
    mvs = rng.standard_normal((nh, s, KK, D), dtype=np.float32)
    mms = np.ones((nh, s, KK), np.uint8)
    bs = (rng.standard_normal((nh, s, s)) * 0.02).astype(np.float32)
    scs = np.full((nh,), np.log(20.0), np.float32)

    # numpy reference for the reduced problem
    def ref():
        qq = qs / np.linalg.norm(qs, axis=-1, keepdims=True)
        kk_ = ks / np.linalg.norm(ks, axis=-1, keepdims=True)
        sc = np.exp(scs)[:, None, None]
        sim = np.einsum("hid,jd->hij", qq, kk_) * sc + bs
        causal = np.triu(np.ones((s, s), bool), 1)
        sim = np.where(causal[None], NEG, sim)
        simm = np.einsum("hid,hijd->hij", qq, mks) * sc
        att = np.concatenate([simm, sim], axis=-1)
        att = att - att.max(-1, keepdims=True)
        att = np.exp(att)
        att = att / att.sum(-1, keepdims=True)
        mem_a, loc_a = att[..., :KK], att[..., KK:]
        return np.einsum("hij,jd->hid", loc_a, vs) + np.einsum(
            "hij,hijd->hid", mem_a, mvs
        )

    bias_s = bs.copy()
    iu = np.triu_indices(P, 1)
    for t in range(nt):
        blk = bias_s[:, t * P : (t + 1) * P, t * P : (t + 1) * P]
        blk[:, iu[0], iu[1]] = NEG
    bias_s = np.exp(bias_s).astype(ml_dtypes.bfloat16)

    nc = build_program(nh, nt)
    sim = CoreSim(nc)
    for name, val in [
        ("q", qs), ("k", ks), ("v", vs), ("mask", ms), ("mem_k", mks),
        ("mem_v", mvs), ("mem_mask", mms), ("bias", bias_s), ("scale", scs),
        ("ident_f", IDENT_F), ("ident_b", IDENT_B),
    ]:
        sim.tensor(name)[:] = val
    sim.simulate()
    got = np.array(sim.tensor("out")).reshape(nh, s, D)
    exp = ref()
    err = np.abs(got - exp).max() / np.abs(exp).max()
    print("abs-rel err:", err)
    assert err < 2e-2, err
    print("CoreSim smoke PASSED")



# revision 14
# speedup vs baseline: 2.4800x; 2.4800x over previous
"""KNN-Attention Trainium2 kernel (Bass/Tile), SPMD over 8 NeuronCores.

Problem (nn_KNNAttention): B=2, H=8, S=2048, D=64, K=32.
  q:[B,H,S,D] k,v:[B,S,D] mask:[B,S] mem_k,mem_v:[B,H,S,K,D]
  mem_mask:[B,H,S,K] rel_pos_bias:[1,H,S,S] scale:[H,1,1]
  out[b,h,i,:] = softmax([sim_mem | sim_local]) @ [mem_v | v]

Sharding: data-parallel over B x tensor-parallel over H.
core c -> (b = c//4, heads 2*(c%4), 2*(c%4)+1). k/v replicated per b.

Host prep (numpy, per-tensor only -- no cross-tensor contractions):
  qhat = l2norm(q)*exp(scale) bf16; qT = qhat^T; kT = l2norm(k)^T bf16;
  v' = [v*mask | mask] bf16; mem_k bf16; mem_v -> [S, D+1, K] bf16 with a
  ones-row at d=D (gives the mem denominator from the same reduce);
  bias -> per-(head,i-tile) packed [j,i]-transposed blocks, causal upper
  triangle of diagonal blocks = -1e30; mem_mask -> additive 0/-1e30 bf16.
  Everything per-tile is packed into ONE dram blob so each (h,it) needs a
  single input DMA (SP sequencer pays ~565ns per DMA).

Device dataflow per (head, i-tile of 128 tokens):
  Local (no stabilizer -- logits <= ~21 so exp fits fp32/bf16 easily):
    per 4 j-blocks: psum[j,512] = ident@biasT (PE) then += kT_jb^T@qT_it
    (PE, bf16); ACT exp psum -> bf16; AV psum[i,65] += expT_jb^T @ v'_jb
    accumulated over all j<=i blocks. Column 65 of v' is the mask, so
    psum[:,64] is the local softmax denominator.
  Mem: prod = mem_k * qhat (GPSIMD, bf16); d-reduce via DVE bf16 2x
    tree-adds + small reduce -> simmem f32; +mmadd; rowmax -> M =
    max(rowmax,21); expmem = exp(simmem-M) (ACT, bf16); prod2 = memvT *
    expmem (DVE 2x); k-reduce tree -> [memout | zmem] f32.
  Combine: num65 = psum_av * exp(-M) + memnum65 (one fused DVE stt);
    out = num65[:,:64] * recip(num65[:,64]).
"""

import os
import sys

import numpy as np

sys.path.insert(0, "/opt/trn_rl_repo")

import concourse.bass as bass
import concourse.mybir as mybir
import concourse.tile as tile
from concourse import bacc
from concourse.bass_utils import run_bass_kernel_spmd
from contextlib import ExitStack

import ml_dtypes

B, H, S, D, KK = 2, 8, 2048, 64, 32
P = 128
NT = S // P  # 16 i-tiles
NH = 2  # heads per core
N_CORES = 8
NEGBIG = -1.0e30

BF16 = mybir.dt.bfloat16
F32 = mybir.dt.float32
AX = mybir.AxisListType
ALU = mybir.AluOpType
ACTF = mybir.ActivationFunctionType

IDENT_B = np.eye(P, dtype=np.float32).astype(ml_dtypes.bfloat16)


def _register_mult_cumsum():
    """Custom DVE op: out[p, k] = cumsum_k(in0[p,k] * in1[p,k]) in fp32.

    Fuses the mem-branch multiply with its segmented reduce: segment sums
    are recovered from differences of segment-end prefix values. One DVE
    pass (1 elem/cycle, fp32 state) replaces mult + tree-adds + reduce and
    removes all fp16 rounding past the inputs. Registered at runtime via
    the per-NEFF custom-DVE table (designed extension point; no firmware
    change)."""
    from concourse import dve_ops as dvo
    from concourse.dve_spec import Spec, Src0, Src1, AluOp, scan, lower
    from concourse.dve_uop import DveOpSpec

    name = "MULT_CUMSUM_ANT"
    for o in dvo.OPS:
        if o.name == name:
            return o

    def _ref(in0, in1, s0, s1, imm2):
        a = np.asarray(in0, np.float32)
        b = np.asarray(in1, np.float32)
        p = a.reshape(a.shape[0], -1) * b.reshape(b.shape[0], -1)
        return np.add.accumulate(p, axis=-1)

    spec = Spec(body=scan(AluOp.ADD, Src0 * Src1), reference=_ref)
    row = dvo._CUSTOM_DVE_ROW_BASE + len(dvo.OPS)
    shas = {}
    for ver in ("v3", "v4"):
        uops = lower(spec, ver=ver)
        shas[ver] = DveOpSpec(name=name, opcode=row, uops=uops, rd1_en=True).sha(ver)
    op = dvo.DveOp(name, spec, subdim=False, uops_sha=shas)
    dvo.OPS.append(op)
    dvo.CUSTOM_DVE_SPECS[name] = spec
    dvo._SUB_OPCODE_FOR_NAME[name] = row
    return op


MULT_CUMSUM = _register_mult_cumsum()

# per-tile record: [qhat 64 | memk KK*D | memvT (D+1)*KK | mmadd KK | biasT (it+1)*P]
REC_FIX = D + KK * D + (D + 1) * KK + KK  # 4224


def record_layout(nt):
    """[(offset, reclen)] per i-tile within one head's blob region + total."""
    offs = []
    off = 0
    for it in range(nt):
        reclen = REC_FIX + (it + 1) * P
        offs.append((off, reclen))
        off += reclen
    return offs, off


# Fraction of the mem_k*q multiply that runs on GPSIMD (rest on DVE).
# GPSIMD shares SBUF ports with the DVE, so offloading streams to it mostly
# serializes with DVE work -- 0 keeps everything on the (2x-mode) DVE.
KG_GP = int(os.environ.get("KG_GP", "0"))


def build_program(nh=NH, nt=NT):
    nc = bacc.Bacc("TRN2")
    s = nt * P
    offs, head_tot = record_layout(nt)

    blob_d = nc.dram_tensor("blob", [P, nh * head_tot], BF16, kind="ExternalInput")
    kT_d = nc.dram_tensor("kT", [D, s], F16, kind="ExternalInput")
    qT_d = nc.dram_tensor("qT", [nh, D, s], F16, kind="ExternalInput")
    vp_d = nc.dram_tensor("vp", [s, D + 1], BF16, kind="ExternalInput")
    ident_d = nc.dram_tensor("ident_b", [P, P], BF16, kind="ExternalInput")
    out_d = nc.dram_tensor("out", [nh, s, D], F32, kind="ExternalOutput")

    recmax = REC_FIX + nt * P

    with tile.TileContext(nc) as tc, ExitStack() as ctx:
        const = ctx.enter_context(tc.tile_pool(name="const", bufs=1))
        qpool = ctx.enter_context(tc.tile_pool(name="qpool", bufs=2))
        stream = ctx.enter_context(tc.tile_pool(name="stream", bufs=3))
        memw = ctx.enter_context(tc.tile_pool(name="memw", bufs=3))
        smallw = ctx.enter_context(tc.tile_pool(name="smallw", bufs=4))
        expp = ctx.enter_context(tc.tile_pool(name="expp", bufs=4))
        outp = ctx.enter_context(tc.tile_pool(name="outp", bufs=2))
        ps_sco = ctx.enter_context(tc.tile_pool(name="ps_sco", bufs=3, space="PSUM"))
        ps_u = ctx.enter_context(tc.tile_pool(name="ps_u", bufs=2, space="PSUM"))

        ident = const.tile([P, P], BF16)
        nc.sync.dma_start(out=ident, in_=ident_d[:])
        kT = const.tile([D, s], F16)
        nc.sync.dma_start(out=kT, in_=kT_d[:])
        vp = const.tile([P, nt, D + 1], BF16)
        nc.sync.dma_start(out=vp, in_=vp_d[:].rearrange("(t p) c -> p t c", p=P))

        for h in range(nh):
            qTh = qpool.tile([D, s], F16, tag="qTh")
            nc.sync.dma_start(out=qTh, in_=qT_d[h])
            outacc = outp.tile([P, nt, D], F32, tag="outacc")

            for it in range(nt):
                jext = (it + 1) * P
                nblk = it + 1
                off, reclen = offs[it]
                off += h * head_tot

                rec = stream.tile([P, recmax], BF16, tag="rec")
                nc.sync.dma_start(
                    out=rec[:, 0:reclen], in_=blob_d[:, off : off + reclen]
                )
                qhat_t = rec[:, 0:D]
                memk_t = rec[:, D : D + KK * D].rearrange("p (k d) -> p k d", k=KK)
                o2 = D + KK * D
                memvT_t = rec[:, o2 : o2 + (D + 1) * KK].rearrange(
                    "p (d k) -> p d k", d=D + 1
                )
                o3 = o2 + (D + 1) * KK
                mmadd_t = rec[:, o3 : o3 + KK]
                o4 = o3 + KK
                biasT_t = rec[:, o4 : o4 + jext]

                # ---- mem branch: sim = sum_d mem_k * qhat ----
                prod = memw.tile([P, KK, D], BF16, tag="prod")
                qbc = qhat_t[:, None, :]
                if KG_GP > 0:
                    nc.gpsimd.tensor_tensor(
                        prod[:, 0:KG_GP, :],
                        memk_t[:, 0:KG_GP, :],
                        qbc.to_broadcast((P, KG_GP, D)),
                        ALU.mult,
                    )
                if KG_GP < KK:
                    nc.vector.tensor_tensor(
                        prod[:, KG_GP:KK, :],
                        memk_t[:, KG_GP:KK, :],
                        qbc.to_broadcast((P, KK - KG_GP, D)),
                        ALU.mult,
                    )
                t32 = treew.tile([P, KK, 32], BF16, tag="t32")
                nc.vector.tensor_add(t32, prod[:, :, 0:32], prod[:, :, 32:64])
                t16 = treew.tile([P, KK, 16], BF16, tag="t16")
                nc.vector.tensor_add(t16, t32[:, :, 0:16], t32[:, :, 16:32])
                t8 = treew.tile([P, KK, 8], BF16, tag="t8")
                nc.vector.tensor_add(t8, t16[:, :, 0:8], t16[:, :, 8:16])
                simmem = smallw.tile([P, KK], F32, tag="simmem")
                nc.vector.tensor_reduce(simmem, t8, axis=AX.X, op=ALU.add)
                # masked sim (mmadd is 0 / -1e30), rowmax, M = max(rowmax, 21)
                simm_m = smallw.tile([P, KK], F32, tag="simm_m")
                nc.vector.tensor_add(simm_m, simmem, mmadd_t)
                rowmax = smallw.tile([P, 1], F32, tag="rowmax")
                nc.vector.tensor_reduce(rowmax, simm_m, axis=AX.X, op=ALU.max)
                negM = smallw.tile([P, 1], F32, tag="negM")
                nc.vector.tensor_scalar(negM, rowmax, 21.0, -1.0, ALU.max, ALU.mult)
                expmem = smallw.tile([P, KK], BF16, tag="expmem")
                nc.scalar.activation(expmem, simm_m, ACTF.Exp, bias=negM)
                eM = smallw.tile([P, 1], F32, tag="eM")
                nc.scalar.activation(eM, negM, ACTF.Exp)

                # prod2 = memvT * expmem  (row d=D of memvT is ones -> zmem)
                prod2 = memw.tile([P, D + 1, KK], BF16, tag="prod2")
                nc.vector.tensor_tensor(
                    prod2,
                    memvT_t,
                    expmem[:, None, :].to_broadcast((P, D + 1, KK)),
                    ALU.mult,
                )
                e16 = treew.tile([P, D + 1, 16], BF16, tag="e16")
                nc.vector.tensor_add(e16, prod2[:, :, 0:16], prod2[:, :, 16:32])
                e8 = treew.tile([P, D + 1, 8], BF16, tag="e8")
                nc.vector.tensor_add(e8, e16[:, :, 0:8], e16[:, :, 8:16])
                e4 = treew.tile([P, D + 1, 4], BF16, tag="e4")
                nc.vector.tensor_add(e4, e8[:, :, 0:4], e8[:, :, 4:8])
                memnum = smallw.tile([P, D + 1], F32, tag="memnum")
                nc.vector.tensor_reduce(memnum, e4, axis=AX.X, op=ALU.add)

                # ---- local branch ----
                psum_u = ps_u.tile([P, D + 1], F32, tag="u")
                for g in range((nblk + 3) // 4):
                    b0 = 4 * g
                    bn = min(4, nblk - b0)
                    w = bn * P
                    ps = ps_sco.tile([P, 4 * P], F32, tag="sco")
                    nc.tensor.matmul(
                        ps[:, 0:w],
                        lhsT=ident,
                        rhs=biasT_t[:, b0 * P : b0 * P + w],
                        start=True,
                        stop=False,
                    )
                    for jb in range(b0, b0 + bn):
                        nc.tensor.matmul(
                            ps[:, (jb - b0) * P : (jb - b0 + 1) * P],
                            lhsT=kT[:, jb * P : (jb + 1) * P],
                            rhs=qTh[:, it * P : (it + 1) * P],
                            start=False,
                            stop=(jb == b0 + bn - 1),
                        )
                    expb = expp.tile([P, 4 * P], BF16, tag="expb")
                    nc.scalar.activation(expb[:, 0:w], ps[:, 0:w], ACTF.Exp)
                    for jb in range(b0, b0 + bn):
                        nc.tensor.matmul(
                            psum_u,
                            lhsT=expb[:, (jb - b0) * P : (jb - b0 + 1) * P],
                            rhs=vp[:, jb, :],
                            start=(jb == 0),
                            stop=(jb == it),
                        )

                # ---- combine ----
                num65 = smallw.tile([P, D + 1], F32, tag="num65")
                nc.vector.scalar_tensor_tensor(
                    num65, psum_u, eM, memnum, op0=ALU.mult, op1=ALU.add
                )
                rz = smallw.tile([P, 1], F32, tag="rz")
                nc.vector.reciprocal(rz, num65[:, D : D + 1])
                nc.scalar.mul(outacc[:, it, :], num65[:, 0:D], rz)

            nc.sync.dma_start(
                out=out_d[h].rearrange("(t p) d -> p t d", p=P), in_=outacc
            )

    nc.compile()
    return nc


def _l2norm(t):
    return t / np.maximum(np.linalg.norm(t, axis=-1, keepdims=True), 1e-12)


def _bf(x):
    return np.ascontiguousarray(x.astype(ml_dtypes.bfloat16))


def pack_core(qhat_bf, memk_bf, memvT_bf, mmadd_bf, biasTp, nh, nt):
    """Build the [P, nh*head_tot] blob for one core.

    qhat_bf [nh,s,D]; memk_bf [nh,s,KK,D]; memvT_bf [nh,s,D+1,KK];
    mmadd_bf [nh,s,KK]; biasTp [nh,P,head_bias_tot] (packed transposed bias).
    """
    offs, _ = record_layout(nt)
    segs = []
    boff = 0
    for h in range(nh):
        boff = 0
        for it in range(nt):
            r = slice(it * P, (it + 1) * P)
            jext = (it + 1) * P
            segs.append(qhat_bf[h, r].reshape(P, D))
            segs.append(memk_bf[h, r].reshape(P, KK * D))
            segs.append(memvT_bf[h, r].reshape(P, (D + 1) * KK))
            segs.append(mmadd_bf[h, r].reshape(P, KK))
            segs.append(biasTp[h][:, boff : boff + jext])
            boff += jext
    return np.ascontiguousarray(np.concatenate(segs, axis=1))


def pack_biasT(bias_f32, nt):
    """bias [nh, s, s] f32 -> [nh, P, sum_it (it+1)*P] bf16, [j,i]-transposed
    per 128-block, causal -1e30 merged into diagonal blocks."""
    nh = bias_f32.shape[0]
    tot = sum((it + 1) * P for it in range(nt))
    out = np.empty((nh, P, tot), dtype=ml_dtypes.bfloat16)
    iu = np.triu_indices(P, 1)
    for h in range(nh):
        boff = 0
        for it in range(nt):
            for jb in range(it + 1):
                blk = bias_f32[
                    h, it * P : (it + 1) * P, jb * P : (jb + 1) * P
                ].T.copy()  # [jp, ip]
                if jb == it:
                    blk[iu[0], iu[1]] = 0.0
                    blk[iu[0], iu[1]] = NEGBIG  # jp > ip -> masked
                out[h, :, boff : boff + P] = blk.astype(ml_dtypes.bfloat16)
                boff += P
    return out


_CACHED = {}
TRACE = False
TRACE_CORES = [0]
STITCH = False
LAST_RESULTS = None


def _get_program(nh=NH, nt=NT):
    key = (nh, nt)
    if key not in _CACHED:
        _CACHED[key] = build_program(nh, nt)
    return _CACHED[key]


def kernel(**inputs):
    q = np.asarray(inputs["q"], dtype=np.float32)
    k = np.asarray(inputs["k"], dtype=np.float32)
    v = np.asarray(inputs["v"], dtype=np.float32)
    mask = np.asarray(inputs["mask"], dtype=np.float32)
    mem_k = np.asarray(inputs["mem_k"], dtype=np.float32)
    mem_v = np.asarray(inputs["mem_v"], dtype=np.float32)
    mem_mask = np.asarray(inputs["mem_mask"]).astype(bool)
    rel_pos_bias = np.asarray(inputs["rel_pos_bias"], dtype=np.float32)
    scale = np.asarray(inputs["scale"], dtype=np.float32).reshape(H)

    esc = np.exp(scale)  # [H]
    qhat = _l2norm(q) * esc[None, :, None, None]  # [B,H,S,D]
    qhat_bf = _bf(qhat)
    qT_bf = np.ascontiguousarray(np.swapaxes(qhat, 2, 3).astype(np.float16))  # [B,H,D,S]
    kT_bf = np.ascontiguousarray(np.swapaxes(_l2norm(k), 1, 2).astype(np.float16))  # [B,D,S]
    vp = np.concatenate([v * mask[:, :, None], mask[:, :, None]], axis=2)  # [B,S,65]
    vp_bf = _bf(vp)
    memk_bf = _bf(mem_k)  # [B,H,S,KK,D]
    memvT = np.concatenate(
        [np.swapaxes(mem_v, 3, 4), np.ones((B, H, S, 1, KK), np.float32)], axis=3
    )  # [B,H,S,D+1,KK]
    memvT_bf = _bf(memvT)
    mmadd_bf = _bf(np.where(mem_mask, 0.0, NEGBIG))  # [B,H,S,KK]
    bias = rel_pos_bias.reshape(H, S, S)

    nc = _get_program()
    biasTp_h = {}
    in_maps = []
    for c in range(N_CORES):
        b = c // 4
        h0 = 2 * (c % 4)
        if h0 not in biasTp_h:
            biasTp_h[h0] = pack_biasT(bias[h0 : h0 + NH], NT)
        blob = pack_core(
            qhat_bf[b, h0 : h0 + NH],
            memk_bf[b, h0 : h0 + NH],
            memvT_bf[b, h0 : h0 + NH],
            mmadd_bf[b, h0 : h0 + NH],
            biasTp_h[h0],
            NH,
            NT,
        )
        in_maps.append(
            {
                "blob": blob,
                "kT": kT_bf[b],
                "qT": np.ascontiguousarray(qT_bf[b, h0 : h0 + NH]),
                "vp": vp_bf[b],
                "ident_b": IDENT_B,
            }
        )

    global LAST_RESULTS
    kwargs = {}
    if TRACE:
        kwargs.update(trace=True, trace_cores=TRACE_CORES, stitch_traces=STITCH)
    res = run_bass_kernel_spmd(nc, in_maps, core_ids=list(range(N_CORES)), **kwargs)
    LAST_RESULTS = res

    out = np.zeros((B, H, S, D), np.float32)
    for c in range(N_CORES):
        b = c // 4
        h0 = 2 * (c % 4)
        out[b, h0 : h0 + NH] = res.results[c]["out"]
    return out


if __name__ == "__main__":
    # CoreSim smoke test on a reduced config
    from concourse.bass_interp import CoreSim

    nh = int(os.environ.get("SMOKE_NH", "1"))
    nt = int(os.environ.get("SMOKE_NT", "2"))
    s = nt * P
    rng = np.random.default_rng(0)
    qs = rng.standard_normal((nh, s, D)).astype(np.float32)
    ks = rng.standard_normal((s, D)).astype(np.float32)
    vs = rng.standard_normal((s, D)).astype(np.float32)
    ms = np.ones((s,), np.float32)
    mks = rng.standard_normal((nh, s, KK, D)).astype(np.float32)
    mvs = rng.standard_normal((nh, s, KK, D)).astype(np.float32)
    mms = np.ones((nh, s, KK), bool)
    mms[:, 5, 7] = False  # exercise mem_mask
    bs = (rng.standard_normal((nh, s, s)) * 0.02).astype(np.float32)
    scs = np.full((nh,), np.log(20.0), np.float32)

    def ref():
        qq = qs / np.linalg.norm(qs, axis=-1, keepdims=True)
        kk_ = ks / np.linalg.norm(ks, axis=-1, keepdims=True)
        sc = np.exp(scs)[:, None, None]
        sim = np.einsum("hid,jd->hij", qq, ks_n := kk_) * sc + bs
        causal = np.triu(np.ones((s, s), bool), 1)
        sim = np.where(causal[None], -np.finfo(np.float32).max, sim)
        simm = np.einsum("hid,hikd->hik", qq, mks) * sc
        simm = np.where(mms, simm, -np.finfo(np.float32).max)
        att = np.concatenate([simm, sim], axis=-1)
        att = att - att.max(-1, keepdims=True)
        att = np.exp(att)
        att = att / att.sum(-1, keepdims=True)
        mem_a, loc_a = att[..., :KK], att[..., KK:]
        return np.einsum("hij,jd->hid", loc_a, vs) + np.einsum(
            "hik,hikd->hid", mem_a, mvs
        )

    esc = np.exp(scs)
    qhat = qs / np.maximum(np.linalg.norm(qs, axis=-1, keepdims=True), 1e-12)
    qhat = qhat * esc[:, None, None]
    qhat_bf = _bf(qhat)
    qT_bf = np.ascontiguousarray(np.swapaxes(qhat, 1, 2).astype(np.float16))
    khat = ks / np.maximum(np.linalg.norm(ks, axis=-1, keepdims=True), 1e-12)
    kT_bf = np.ascontiguousarray(khat.T.astype(np.float16))
    vp_bf = _bf(np.concatenate([vs * ms[:, None], ms[:, None]], axis=1))
    memk_bf = _bf(mks)
    memvT_bf = _bf(
        np.concatenate(
            [np.swapaxes(mvs, 2, 3), np.ones((nh, s, 1, KK), np.float32)], axis=2
        )
    )
    mmadd_bf = _bf(np.where(mms, 0.0, NEGBIG))
    biasTp = pack_biasT(bs, nt)
    blob = pack_core(qhat_bf, memk_bf, memvT_bf, mmadd_bf, biasTp, nh, nt)

    nc = build_program(nh, nt)
    sim = CoreSim(nc)
    for name, val in [
        ("blob", blob),
        ("kT", kT_bf),
        ("qT", qT_bf),
        ("vp", vp_bf),
        ("ident_b", IDENT_B),
    ]:
        sim.tensor(name)[:] = val
    sim.simulate()
    got = np.array(sim.tensor("out")).reshape(nh, s, D)
    exp = ref()
    err = np.abs(got - exp).max() / np.abs(exp).max()
    print("abs-rel err:", err)
    assert err < 2e-2, err
    print("CoreSim smoke PASSED")
